# revision 13
# baseline (speedup 1.0000x reference)
"""Trainium2 Bass kernel for nn_ContrastiveLoss (8-core data-parallel).

Contract: kernel(**inputs) takes the FULL unsharded inputs
(feats1 [2048,512] f32, feats2 [2048,512] f32, overlap_inds [8] i32, bs=256)
and returns the full output (acc, loss) like the reference.

Math restructuring (see reference):
  feats = concat(feats1, feats2)  [N=4096, F=512]
  G = feats @ feats.T ; sim = exp(TEMP*G) ; log(sim) = TEMP*G
  labels are constant on 16 blocks of 256 consecutive rows, so every mask
  (same / pos / neg / cross) is block-constant (minus the diagonal).  Each
  label appears in at most two blocks (one per half), so each row has one
  "self" positive block and at most one "partner" positive block.

  Per row i:   negsum_i = sum_{neg blocks} rowsum(e)
               thr_i    = max_{neg blocks} rowmax(e)
               count_i  = #{pos j : e_ij > thr_i}   (acc numerator)
               lossnum_i = PW_i*log(negsum_i) - TEMP*sum_{pos} cross*G_ij

Device computes, per core (rows of 2 groups), streaming over column tiles:
  e = exp(TEMP*G) with fused per-256-block row-sums (ScalarE accum_out) and
  per-block row-max (VectorE).  The first 512 permuted columns (= the two
  candidate positive blocks) of e are written out ("pose").  Everything else
  is assembled on the host from the tiny per-block stats; borderline count
  rows are refined exactly on the host from feats.

Sharding: core c owns row groups {c, 8+c}.  Host hands each core featsT with
columns permuted to [block c, block 8+c, remaining 14 blocks], so one uniform
SPMD NEFF serves all cores (lhsT = first 512 permuted columns; pose = first
512 columns of each row band).  Inputs are replicated (8 MB/core) -> no
collectives; scalar reduction happens on the host.
"""

import os
import sys

sys.path.insert(0, "/opt/trn_rl_repo")
# this container has no NTFF trace hook (antenv is a stub); make sure a
# stray BASS_TRACE env can never route us onto that path
os.environ["BASS_NEVER_TRACE"] = "1"

from contextlib import ExitStack

import numpy as np

import concourse.mybir as mybir
import concourse.tile as tile
from concourse import bacc
from concourse.bass_utils import run_bass_kernel_spmd

TEMP = 0.02
OTHERWEIGHT = 0.5

NCORES = 8
N = 4096          # total rows (feats1 + feats2)
F = 512           # feature dim
BS = 256          # rows per group/block
NBLK = 16         # 256-row blocks
ROWS_PER_CORE = 512
MTILES = 4        # 128-row subtiles per core
NTILES = 8        # 512-col tiles per row band
KT = 4            # 128-row contraction tiles of F

_BUILT = None     # cached (nc,) build
_LAST_RESULTS = None


def _build_nc():
    """Build the uniform SPMD Tile kernel (one NEFF for all 8 cores)."""
    f32 = mybir.dt.float32
    f32r = mybir.dt.float32r

    nc = bacc.Bacc("TRN2", target_bir_lowering=False, debug=False)
    ft_d = nc.dram_tensor("ft", [F, N], f32r, kind="ExternalInput")
    pose_d = nc.dram_tensor("pose", [MTILES, 128, 512], f32, kind="ExternalOutput")
    # stats per m-subtile (20 cols): [0:2] per-block e-sums of tile 0,
    # [2:9] pair e-sums of tiles 1..7, [10:12] per-block e-maxes of tile 0,
    # [12:19] pair e-maxes of tiles 1..7.  Tiles 1..7 are all-negative for
    # every row group (the permutation puts both positive candidates in
    # tile 0), so pair granularity suffices there.
    stat_d = nc.dram_tensor("stat", [128, MTILES * 20], f32, kind="ExternalOutput")

    Exp = mybir.ActivationFunctionType.Exp

    with tile.TileContext(nc) as tc, ExitStack() as ctx:
        ftp = ctx.enter_context(tc.tile_pool(name="ft", bufs=1))
        posp = ctx.enter_context(tc.tile_pool(name="pose", bufs=1))
        ep = ctx.enter_context(tc.tile_pool(name="e", bufs=12))
        dp = ctx.enter_context(tc.tile_pool(name="dummy", bufs=2))
        statp = ctx.enter_context(tc.tile_pool(name="stat", bufs=1))
        psp = ctx.enter_context(tc.tile_pool(name="ps", bufs=8, space="PSUM"))

        ft_t = [ftp.tile([128, N], f32r, name=f"ft{kf}", tag=f"ft{kf}") for kf in range(KT)]
        # stream the input in 512-col chunks (n-tile granularity) so PE can
        # start as soon as the first MB lands
        for q in range(NTILES):
            for kf in range(KT):
                nc.sync.dma_start(
                    ft_t[kf][:, q * 512 : (q + 1) * 512],
                    ft_d.ap()[kf * 128 : (kf + 1) * 128, q * 512 : (q + 1) * 512],
                )

        pose_t = [posp.tile([128, 512], f32, name=f"pose{m}", tag=f"pose{m}") for m in range(MTILES)]
        stat_t = statp.tile([128, MTILES * 20], f32, tag="stat")

        for n in range(NTILES):
            for m in range(MTILES):
                ps = psp.tile([128, 512], f32, name="ps", tag="ps")
                for kf in range(KT):
                    nc.tensor.matmul(
                        ps[:],
                        ft_t[kf][:, m * 128 : (m + 1) * 128],
                        ft_t[kf][:, n * 512 : (n + 1) * 512],
                        start=(kf == 0),
                        stop=(kf == KT - 1),
                    )
                base = m * 20
                et = pose_t[m] if n == 0 else ep.tile([128, 512], f32, name="et", tag="e")
                if n == 0:
                    # tile 0: per-block sums (2 fused exp+accum) + per-block max
                    for h in range(2):
                        nc.scalar.activation(
                            et[:, h * 256 : (h + 1) * 256],
                            ps[:, h * 256 : (h + 1) * 256],
                            Exp,
                            scale=TEMP,
                            accum_out=stat_t[:, base + h : base + h + 1],
                        )
                    nc.vector.tensor_reduce(
                        stat_t[:, base + 10 : base + 12],
                        et[:].rearrange("p (b x) -> p b x", b=2),
                        axis=mybir.AxisListType.X,
                        op=mybir.AluOpType.max,
                    )
                    nc.sync.dma_start(pose_d.ap()[m], pose_t[m][:])
                else:
                    # tiles 1..7: fused exp+pair-sum on ACT, pair-max on DVE
                    nc.scalar.activation(
                        et[:],
                        ps[:],
                        Exp,
                        scale=TEMP,
                        accum_out=stat_t[:, base + 1 + n : base + 2 + n],
                    )
                    nc.vector.tensor_reduce(
                        stat_t[:, base + 11 + n : base + 12 + n],
                        et[:],
                        axis=mybir.AxisListType.X,
                        op=mybir.AluOpType.max,
                    )

        # two half-stores: m=0,1 stats complete two banks before m=2,3 at
        # n=7, so the first store overlaps the last banks' compute
        nc.sync.dma_start(stat_d.ap()[:, 0:40], stat_t[:, 0:40])
        nc.sync.dma_start(stat_d.ap()[:, 40:80], stat_t[:, 40:80])

    nc.compile()
    return nc


def _labels_np(ov, bs):
    K = ov.shape[0]
    labels1 = np.repeat(np.arange(K), bs)
    non = (ov == 0).astype(np.int64)
    excl = np.cumsum(non) - non
    cls2 = np.where(ov.astype(bool), np.arange(K), K + excl)
    labels2 = np.repeat(cls2, bs)
    return np.concatenate([labels1, labels2])


def kernel(feats1, feats2, overlap_inds, bs):
    global _BUILT, _LAST_RESULTS
    bs = int(bs)
    feats1 = np.asarray(feats1, np.float32)
    feats2 = np.asarray(feats2, np.float32)
    ov = np.asarray(overlap_inds)
    assert feats1.shape == (2048, 512) and feats2.shape == (2048, 512)
    assert bs == BS and ov.shape == (8,)

    feats = np.concatenate([feats1, feats2])              # [N, F]
    featsT = np.ascontiguousarray(feats.T)                # [F, N]
    labels = _labels_np(ov, bs)                           # [N]
    lblock = labels[::BS]                                 # [16] per-block label

    # per-core permuted inputs: blocks [c, 8+c, rest]
    perms = []
    in_maps = []
    for c in range(NCORES):
        pb = [c, 8 + c] + [b for b in range(NBLK) if b not in (c, 8 + c)]
        perms.append(pb)
        cols = np.concatenate([np.arange(b * BS, (b + 1) * BS) for b in pb])
        in_maps.append({"ft": np.ascontiguousarray(featsT[:, cols])})

    if _BUILT is None:
        _BUILT = _build_nc()
    nc = _BUILT

    res = run_bass_kernel_spmd(nc, in_maps, core_ids=list(range(NCORES)))
    _LAST_RESULTS = res

    # ---- host assembly ----
    counts = np.bincount(labels)
    total_pos = float((counts[labels] - 1).sum())

    cnt_rows = np.zeros(N, np.float64)
    lossnum_rows = np.zeros(N, np.float64)
    need_refine = []

    lanes = np.arange(128)
    for c in range(NCORES):
        out = res.results[c]
        pose = out["pose"]                       # [4, 128, 512] f32 (e-domain)
        stat = out["stat"]                       # [128, 128]
        for m in range(MTILES):
            b_self = c if m < 2 else 8 + c
            slot_self = 0 if m < 2 else 1
            b_part = 8 + c if m < 2 else c
            paired = lblock[b_self] == lblock[b_part]

            base = m * 20
            pair_sums = stat[:, base + 2 : base + 9].astype(np.float64)  # [128,7]
            pair_maxs = stat[:, base + 12 : base + 19]                   # [128,7]
            negsum = pair_sums.sum(axis=1)                               # [128]
            thr = pair_maxs.max(axis=1)                                  # [128] f32
            if not paired:  # sibling candidate block is a negative
                negsum = negsum + stat[:, base + (1 - slot_self)].astype(np.float64)
                thr = np.maximum(thr, stat[:, base + 10 + (1 - slot_self)])

            pm = pose[m]                                            # [128, 512]
            sl_self = slice(slot_self * 256, slot_self * 256 + 256)
            sl_part = slice((1 - slot_self) * 256, (1 - slot_self) * 256 + 256)
            diag_col = slot_self * 256 + (m % 2) * 128 + lanes
            e_diag = pm[lanes, diag_col]

            cnt = (pm[:, sl_self] > thr[:, None]).sum(axis=1).astype(np.float64)
            cnt -= (e_diag > thr)
            if paired:
                cnt += (pm[:, sl_part] > thr[:, None]).sum(axis=1)

            g = np.log(pm.astype(np.float64)) / TEMP
            g_diag = g[lanes, diag_col]
            possum = g[:, sl_self].sum(axis=1) - g_diag
            pw = 255.0
            if paired:
                possum += OTHERWEIGHT * g[:, sl_part].sum(axis=1)
                pw += OTHERWEIGHT * 256.0
            lossnum = pw * np.log(negsum) - TEMP * possum

            rows = b_self * BS + (m % 2) * 128 + lanes
            cnt_rows[rows] = cnt
            lossnum_rows[rows] = lossnum

            # borderline rows -> exact host recount (matmul-precision guard)
            thr_g = np.log(thr.astype(np.float64)) / TEMP
            marg = np.abs(g[:, sl_self] - thr_g[:, None])
            marg[lanes, (m % 2) * 128 + lanes] = np.inf  # diagonal isn't pos
            mmin = marg.min(axis=1)
            if paired:
                mmin = np.minimum(mmin, np.abs(g[:, sl_part] - thr_g[:, None]).min(axis=1))
            # also guard the diagonal comparison we subtracted
            mmin = np.minimum(mmin, np.abs(g_diag - thr_g))
            for p in np.nonzero(mmin < 0.25)[0]:
                need_refine.append(rows[p])

    # exact recount of borderline rows, replicating the reference ops
    for i in set(need_refine):
        g_row = feats[i] @ feats.T                       # f32
        sim = np.exp((g_row * np.float32(TEMP)).astype(np.float32))
        negm = labels != labels[i]
        mneg = sim[negm].max()
        posm = labels == labels[i]
        posm[i] = False
        cnt_rows[i] = float((sim[posm] > mneg).sum())

    acc = np.float32(cnt_rows.sum() / total_pos)
    loss = np.float32(lossnum_rows.sum() / total_pos)
    return acc, loss
